# revision 12
# baseline (speedup 1.0000x reference)
"""BitLinear TRN2 kernel: y = x @ W(pweight,nweight)^T + bias.

Sharding: 8 cores = 2 token-shards x 4 out-feature shards. No collectives:
each core preps its full [512, 2048] weight slice locally (the AllGather in
the previous design cost ~67us/call and serialized behind the PE transposes
that feed it).

Per core: xT [2048, 8192] bf16 (token slice, host-transposed + host-cast),
pw/nw [512, 8192] bf16 (out-feature slice, host-cast), y [8192, 512] bf16
(host upcasts to fp32 on gather).

Device pipeline (bf16 compute, fp32 PSUM accumulation), software-pipelined
across reps: weight prep for rep r+1 is emitted one (ob,ch)-unit per x-slab
of rep r's main stage, so the PE transposes slot between matmul groups and
the next rep's matmuls start with at most a ~1us bubble.

  unit(it): SP-queue DMA pw/nw bf16 tile [128 i, 2048 (o n)] (host supplies
      pw/nw pre-transposed to i-major, so no PE transposes are needed)
      -> ACT sigmoid x2 -> DVE sub -> DVE mult by c (cvec) -> DVE reduce
      over n straight into wT[:, it, :]
  slab(sl): Pool-queue (SWDGE) DMA x slab [128, 16it, 512t] bf16; per
      t-tile: 16 accumulating matmuls psum[t 128, o 512] += xs.T @ wT;
      DVE adds bias during PSUM->SBUF (bf16 out); Pool-queue DMA y out.

bias path: bit_ste is an exact identity on the reference's bias_raw values
(k/15 grid), computed host-side along with the tiny cvec constant
(cvec[n] = exps[n]*sigmoid(mask[n])*scale).
"""

import numpy as np

import concourse.bass as bass
import concourse.mybir as mybir
import concourse.tile as tile
from concourse import bacc
from concourse.bass_utils import run_bass_kernel_spmd

N_CORES = 8
R, C = 2, 4  # token shards x out-feature shards
T, I, O, NB = 16384, 2048, 2048, 4
TQ, OC = T // R, O // C  # 8192 tokens, 512 outs per core
P = 128
OCN = OC * NB  # 2048 flattened (o, n) columns of i-major pw/nw
N_IT = I // P  # 16 i-tiles == weight-prep units
TSLAB = 512  # tokens per x slab (4 t-tiles)
N_SLAB = TQ // TSLAB  # 16
VPS = TSLAB // P  # 4 t-tiles per slab
DT = mybir.dt.bfloat16
F32 = mybir.dt.float32

_BUILT = None


def _build_bass(reps=1, mode='full'):
    nc = bacc.Bacc("TRN2", debug=False, num_devices=N_CORES)

    xt_d = nc.dram_tensor("xt", [I, TQ], DT, kind="ExternalInput").ap()
    pw_d = nc.dram_tensor("pw", [I, OCN], DT, kind="ExternalInput").ap()
    nw_d = nc.dram_tensor("nw", [I, OCN], DT, kind="ExternalInput").ap()
    cv_d = nc.dram_tensor("cvec", [P, NB], DT, kind="ExternalInput").ap()
    bias_d = nc.dram_tensor("bias", [P, OC], F32, kind="ExternalInput").ap()
    y_d = nc.dram_tensor("y", [TQ, OC], DT, kind="ExternalOutput").ap()

    xt_r = xt_d.rearrange("(i p) t -> p i t", p=P)  # [128, 16, 8192]
    y_r = y_d.rearrange("(s v p) o -> s p v o", v=VPS, p=P)  # [16, 128, 4, 512]

    with tile.TileContext(nc) as tc:
        with (
            tc.tile_pool(name="const", bufs=1) as const_pool,
            tc.tile_pool(name="wT", bufs=2) as wT_pool,
            tc.tile_pool(name="wio", bufs=2) as wio_pool,
            tc.tile_pool(name="sig", bufs=2) as sig_pool,
            tc.tile_pool(name="soft", bufs=2) as soft_pool,
            tc.tile_pool(name="scl", bufs=2) as scl_pool,
            tc.tile_pool(name="xs", bufs=2) as xs_pool,
            tc.tile_pool(name="yo", bufs=3) as yo_pool,
            tc.tile_pool(name="mm_ps", bufs=4, space="PSUM") as mm_ps,
        ):
            cv_sb = const_pool.tile([P, NB], DT)
            nc.sync.dma_start(cv_sb[:], cv_d[:])
            bias_sb = const_pool.tile([P, OC], F32)
            nc.sync.dma_start(bias_sb[:], bias_d[:])

            def emit_unit(wT_dst, it):
                irow = slice(it * P, (it + 1) * P)
                pwt = wio_pool.tile([P, OCN], DT, tag="pw")
                nc.sync.dma_start(pwt[:], pw_d[irow, :])
                nwt = wio_pool.tile([P, OCN], DT, tag="nw")
                nc.sync.dma_start(nwt[:], nw_d[irow, :])
                if mode == 'dma':
                    return
                sp = sig_pool.tile([P, OCN], DT, tag="sp")
                nc.scalar.activation(
                    sp[:], pwt[:], mybir.ActivationFunctionType.Sigmoid
                )
                sn = sig_pool.tile([P, OCN], DT, tag="sn")
                nc.scalar.activation(
                    sn[:], nwt[:], mybir.ActivationFunctionType.Sigmoid
                )
                soft = soft_pool.tile([P, OCN], DT, tag="soft")
                nc.vector.tensor_sub(out=soft[:], in0=sp[:], in1=sn[:])
                # scaled[i, o, n] = soft * c[n]; wT[i, o] = sum_n
                scaled = scl_pool.tile([P, OCN], DT, tag="scl")
                nc.vector.tensor_tensor(
                    scaled[:].rearrange("p (o n) -> p o n", n=NB),
                    soft[:].rearrange("p (o n) -> p o n", n=NB),
                    cv_sb[:, None, :].to_broadcast((P, OC, NB)),
                    mybir.AluOpType.mult,
                )
                with nc.allow_low_precision(
                    reason="4-element bf16 reduce; well within rel-err budget"
                ):
                    nc.vector.tensor_reduce(
                        wT_dst[:, it, :],
                        scaled[:].rearrange("p (o n) -> p o n", n=NB),
                        axis=mybir.AxisListType.X,
                        op=mybir.AluOpType.add,
                    )

            def emit_slab(wT_src, sl):
                tcols = slice(sl * TSLAB, (sl + 1) * TSLAB)
                xs = xs_pool.tile([P, N_IT, TSLAB], DT, tag="xs")
                nc.gpsimd.dma_start(xs[:], xt_r[:, :, tcols])
                if mode == 'dma':
                    return
                yslab = yo_pool.tile([P, VPS, OC], DT, tag="yo")
                for v in range(VPS):
                    ps = mm_ps.tile([P, OC], F32, tag="ps")
                    for it in range(N_IT):
                        nc.tensor.matmul(
                            ps[:],
                            xs[:, it, v * P : (v + 1) * P],
                            wT_src[:, it, :],
                            start=(it == 0),
                            stop=(it == N_IT - 1),
                        )
                    nc.vector.tensor_tensor(
                        yslab[:, v, :], ps[:], bias_sb[:], mybir.AluOpType.add
                    )
                nc.gpsimd.dma_start(y_r[sl], yslab[:])

            do_w = mode in ('full', 'w', 'dma')
            do_mm = mode in ('full', 'mm')

            # prologue: weights for rep 0
            wT_cur = None
            if do_w:
                wT_cur = wT_pool.tile([P, N_IT, OC], DT, tag="wT")
                for u in range(N_IT):
                    emit_unit(wT_cur, u)
            elif do_mm:
                wT_cur = wT_pool.tile([P, N_IT, OC], DT, tag="wT")
                nc.vector.memset(wT_cur[:], 0)

            for r in range(reps):
                wT_next = None
                if do_w and r + 1 < reps:
                    wT_next = wT_pool.tile([P, N_IT, OC], DT, tag="wT")
                for sl in range(N_SLAB):
                    if wT_next is not None:
                        emit_unit(wT_next, sl)
                    if do_mm or mode == 'dma':
                        emit_slab(wT_cur, sl)
                if wT_next is not None:
                    wT_cur = wT_next

    nc.compile()
    return nc


def get_built():
    global _BUILT
    if _BUILT is None:
        _BUILT = _build_bass()
    return _BUILT


def make_in_maps(
    input, pweight, nweight, exps, bexps, mask_weight, scale, pbias, nbias, biasscale
):
    import ml_dtypes

    bf16 = ml_dtypes.bfloat16
    input = np.asarray(input, dtype=np.float32)
    pweight = np.asarray(pweight, dtype=np.float32)
    nweight = np.asarray(nweight, dtype=np.float32)
    exps = np.asarray(exps, dtype=np.float32)
    bexps = np.asarray(bexps, dtype=np.float32)
    mask_weight = np.asarray(mask_weight, dtype=np.float32)
    scale = np.asarray(scale, dtype=np.float32)
    pbias = np.asarray(pbias, dtype=np.float32)
    nbias = np.asarray(nbias, dtype=np.float32)
    biasscale = np.asarray(biasscale, dtype=np.float32)

    # tiny launch constants, computed exactly as the reference does
    mask = 1.0 / (1.0 + np.exp(-mask_weight))
    c4 = (exps * mask * scale[0]).astype(np.float32)  # [4]
    cvec = np.ascontiguousarray(np.broadcast_to(c4, (P, NB)).astype(bf16))

    bias_raw = (pbias - nbias) @ bexps  # [O]
    step = float(2**NB - 1)
    b = np.clip(bias_raw, -1.0, 1.0)
    bias = (np.round(np.abs(b) * step) / step * np.sign(b)) * biasscale[0]
    bias = bias.astype(np.float32)

    x = input.reshape(T, I)
    xts = [x[tr * TQ : (tr + 1) * TQ].T.astype(bf16) for tr in range(R)]
    # i-major weight layout: [I, (O_c, NB)] so the device needs no transposes
    pws = [
        pweight[oc * OC : (oc + 1) * OC]
        .transpose(1, 0, 2)
        .reshape(I, OCN)
        .astype(bf16)
        for oc in range(C)
    ]
    nws = [
        nweight[oc * OC : (oc + 1) * OC]
        .transpose(1, 0, 2)
        .reshape(I, OCN)
        .astype(bf16)
        for oc in range(C)
    ]
    biases = [
        np.ascontiguousarray(
            np.broadcast_to(bias[oc * OC : (oc + 1) * OC], (P, OC))
        )
        for oc in range(C)
    ]

    in_maps = []
    for core in range(N_CORES):
        tr, oc = divmod(core, C)
        in_maps.append(
            {
                "xt": xts[tr],
                "pw": pws[oc],
                "nw": nws[oc],
                "cvec": cvec,
                "bias": biases[oc],
            }
        )
    return in_maps


def gather_output(results):
    y = np.empty((T, O), dtype=np.float32)
    for core, r in enumerate(results):
        tr, oc = divmod(core, C)
        y[tr * TQ : (tr + 1) * TQ, oc * OC : (oc + 1) * OC] = r["y"].astype(
            np.float32
        )
    return y.reshape(8, T // 8, O)


def kernel(**inputs) -> np.ndarray:
    in_maps = make_in_maps(**inputs)
    nc = get_built()
    res = run_bass_kernel_spmd(nc, in_maps, core_ids=list(range(N_CORES)))
    return gather_output(res.results)
